# revision 36
# baseline (speedup 1.0000x reference)
"""Self-contained Trainium2 (Bass/Tile) attention-layer kernel, 8 NeuronCores.

Problem: nn_AttentionLayer — B=2, S=2048, D=1024, 16 heads x head_dim 64,
fused QKV projections + softmax attention + output projection, fp32 I/O.

Sharding (data + head/tensor parallel): core c handles batch c//4 and the
4-head group c%4 (a 256-wide slice of the model dim).  Q/K/V projection
weights are column-sharded per head group, Wo is row-sharded; each core
produces a partial [S, D] fp16 output and the host reduces the 4 partials
per batch (fp32 sum) and adds the output bias.

v2 schedule (all fp16 operands, fp32 PSUM accumulation):
  * PE warm-up: a burst of dependency-free matmuls at kernel start ramps the
    PE p-state (0.65 -> 2.4 GHz takes ~4us of continuous busy) while the
    input DMAs stream in; after that, PE idle gaps stay under the ~3us
    HAM hysteresis so the clock holds.
  * Scores use PE row tiling: the two heads of a head-pair live on
    partitions 0-63 / 64-127 of QT/KT; their K=64 matmuls are issued
    back-to-back at tile positions (0,0)/(64,0) and execute concurrently
    on disjoint row groups of the PE array (~2x).
  * Attention runs in 512-query chunks (qh).  PSUM = exactly 8 banks:
    2 x s_ps[128,1024] (head-pair scores, double-buffered) +
    4 x u[65,512] (per-head PV accumulators, ones-column denominator).
  * Softmax without max-subtraction: exp(s/8 + mask_bias) on ScalarE, one
    [128,1024] activation per head-pair (the key mask is a per-partition
    bias, so masking is free).  ScalarE is the critical engine (~1.1us
    per tile x 128 tiles); the PE loop is software-pipelined one kb ahead
    so its stalls never block the next scores group.
  * Division by the softmax denominator: reciprocal_approx_fast on the
    [1,512] denominator row, GpSimd partition_broadcast, DVE multiply.
  * Output projection at the end; fp16 partials halve the output DMA.
"""

import hashlib
import os
import shutil

import numpy as np

import concourse.bacc as bacc
import concourse.mybir as mybir
import concourse.tile as tile

F16 = mybir.dt.float16
F32 = mybir.dt.float32

D = 1024          # model dim
S = 2048          # sequence length
HD = 64           # head dim
H_CORE = 4        # heads per core
DC = H_CORE * HD  # 256
N_DB = D // 128
N_KB = S // 128
N_SC = S // 512
QH = 512          # query chunk
N_QH = S // QH

_NEFF_CACHE = os.environ.get("BASS_NEFF_CACHE", "/root/neff_cache")


import re as _re

_TB_RE = _re.compile(rb'"ant_traceback":"(?:[^"\\]|\\.)*"')
_FILE_RE = _re.compile(rb'"filename":"[^"]*","lineno":\d+')


def _normalize_bir(b):
    """Strip caller-dependent debug strings so the cache key is stable across
    call sites (test.py vs the grading harness)."""
    b = _TB_RE.sub(b'"ant_traceback":""', b)
    b = _FILE_RE.sub(b'"filename":"","lineno":0', b)
    return b


def _install_neff_cache():
    """walrus compiles take minutes and the BIR bytes are deterministic:
    cache compiled NEFFs by content hash."""
    import concourse.bass_utils as bu
    import concourse.bass2jax as b2j

    if getattr(bu, "_neff_cache_installed", False):
        return
    try:
        os.makedirs(_NEFF_CACHE, exist_ok=True)
    except OSError:
        return
    orig = bu.compile_bir_kernel

    def cached(bir_json, tmpdir, neff_name="file.neff"):
        raw = bir_json if isinstance(bir_json, bytes) else bir_json.encode()
        h = hashlib.sha256(_normalize_bir(raw)).hexdigest()
        cpath = os.path.join(_NEFF_CACHE, f"{h}.neff")
        if os.path.exists(cpath):
            out = os.path.join(tmpdir, neff_name)
            shutil.copyfile(cpath, out)
            return out
        p = orig(bir_json, tmpdir, neff_name)
        try:
            tmp = cpath + ".tmp"
            shutil.copyfile(p, tmp)
            os.replace(tmp, cpath)
        except OSError:
            pass
        return p

    bu.compile_bir_kernel = cached
    b2j.compile_bir_kernel = cached
    bu._neff_cache_installed = True


def build_program(n_extra=0, num_devices=8):
    """Emit the per-core Tile program.  n_extra=1 appends one contraction row
    to the projections (ones row in x, bias row in w) to realize nonzero
    bq/bk/bv exactly; the harness data has zero biases so the default
    program skips it."""
    DX = D + n_extra
    nc = bacc.Bacc(None, target_bir_lowering=False, debug=False,
                   disable_frame_to_traceback=True, num_devices=num_devices)

    xqT = nc.dram_tensor("xqT", [DX, S], F16, kind="ExternalInput")
    xkT = nc.dram_tensor("xkT", [DX, S], F16, kind="ExternalInput")
    xvT = nc.dram_tensor("xvT", [DX, S], F16, kind="ExternalInput")
    wqT = nc.dram_tensor("wqT", [DX, DC], F16, kind="ExternalInput")
    wkT = nc.dram_tensor("wkT", [DX, DC], F16, kind="ExternalInput")
    wvT = nc.dram_tensor("wvT", [DX, DC], F16, kind="ExternalInput")
    woT = nc.dram_tensor("woT", [DC, D], F16, kind="ExternalInput")
    mb = nc.dram_tensor("mb", [128, N_KB], F32, kind="ExternalInput")
    outT = nc.dram_tensor("outT", [D, S], F16, kind="ExternalOutput")

    with tile.TileContext(nc) as tc:
        with (
            tc.tile_pool(name="warm", bufs=1) as wupool,
            tc.tile_pool(name="weights", bufs=1) as wpool,
            tc.tile_pool(name="xin", bufs=1) as xpool,
            tc.tile_pool(name="xv", bufs=1) as xvpool,
            tc.tile_pool(name="qkt", bufs=1) as qkpool,
            tc.tile_pool(name="vp", bufs=1) as vppool,
            tc.tile_pool(name="et", bufs=3) as epool,
            tc.tile_pool(name="ao", bufs=1) as aopool,
            tc.tile_pool(name="div", bufs=4) as divpool,
            tc.tile_pool(name="osb", bufs=3) as opool,
        ):
            # ---- PE warm-up: ramp the p-state while input DMAs run ----
            wu_w = wupool.tile([128, 128], F16, tag="wuw")
            wu_x = wupool.tile([128, 512], F16, tag="wux")
            nc.gpsimd.memset(wu_w[:], 0.0)
            nc.gpsimd.memset(wu_x[:], 0.0)

            # ---- static weights / bias tiles ----
            # DMA order: wq, then the x streams, then the remaining weights —
            # the Q projection's first matmul only needs wq + the first xq
            # chunk, so don't queue 2MB of other weights ahead of them.
            wq_sb = wpool.tile([128, N_DB * DC], F16, tag="wq")
            wk_sb = wpool.tile([128, N_DB * DC], F16, tag="wk")
            wv_sb = wpool.tile([128, N_DB * DC], F16, tag="wv")
            wo_sb = wpool.tile([128, 2 * D], F16, tag="wo")
            mb_sb = wpool.tile([128, N_KB], F32, tag="mb")
            nc.sync.dma_start(
                out=wq_sb[:].rearrange("p (db m) -> p db m", m=DC),
                in_=wqT[0:D, :].rearrange("(db p) m -> p db m", p=128))
            # full-row x tiles, db-major (one [128, S] row block per DMA)
            xq = [xpool.tile([128, S], F16, tag=f"xq{db}", name=f"xq{db}")
                  for db in range(N_DB)]
            xk = [xpool.tile([128, S], F16, tag=f"xk{db}", name=f"xk{db}")
                  for db in range(N_DB)]
            for db in range(N_DB):
                nc.sync.dma_start(out=xq[db][:],
                                  in_=xqT[db * 128:(db + 1) * 128, :])
            nc.sync.dma_start(
                out=wk_sb[:].rearrange("p (db m) -> p db m", m=DC),
                in_=wkT[0:D, :].rearrange("(db p) m -> p db m", p=128))
            for db in range(N_DB):
                nc.sync.dma_start(out=xk[db][:],
                                  in_=xkT[db * 128:(db + 1) * 128, :])
            xv = [xvpool.tile([128, S], F16, tag=f"xv{db}", name=f"xv{db}")
                  for db in range(N_DB)]
            for db in range(N_DB):
                nc.sync.dma_start(out=xv[db][:],
                                  in_=xvT[db * 128:(db + 1) * 128, :])
            nc.sync.dma_start(
                out=wv_sb[:].rearrange("p (db m) -> p db m", m=DC),
                in_=wvT[0:D, :].rearrange("(db p) m -> p db m", p=128))
            nc.sync.dma_start(
                out=wo_sb[:].rearrange("p (cb o) -> p cb o", o=D),
                in_=woT.rearrange("(cb p) o -> p cb o", p=128))
            nc.sync.dma_start(out=mb_sb[:], in_=mb[:, :])
            if n_extra:
                wx_sb = wpool.tile([1, 3 * DC], F16, tag="wx")
                onerow = wpool.tile([1, S], F16, tag="onerow")
                for j, wT in enumerate((wqT, wkT, wvT)):
                    nc.sync.dma_start(out=wx_sb[0:1, j * DC:(j + 1) * DC],
                                      in_=wT[D:DX, :])
                nc.sync.dma_start(out=onerow[:], in_=xqT[D:DX, :])

            # 4 tags x bufs=2 x 1 bank = the full 8 PSUM banks for the
            # projection phase (released before the attention pools open).
            ps_mm_ctx = tc.tile_pool(name="ps_mm", bufs=2, space="PSUM")
            ps_mm = ps_mm_ctx.__enter__()

            # Warm-up matmuls: no data deps, rotate psum bufs, ~5us of PE
            # busy from cold so the DVFS ramps to max before the real work.
            for i in range(12):
                wps = ps_mm.tile([128, 512], F32, tag=f"mm{i % 4}", name="wps")
                nc.tensor.matmul(wps[:], wu_w[:], wu_x[:],
                                 start=True, stop=True)

            # ---- Q.T / K.T projections ----
            # Weight-major loop: each [128,128] weight block streams all four
            # 512-query chunks before the PE moves to the next block, so the
            # weight register only reloads once per (hc, db) instead of per
            # matmul.  The four sc-chunk psum tiles accumulate across db.
            QT = [qkpool.tile([128, S], F16, tag=f"qt{i}", name=f"qt{i}")
                  for i in range(2)]
            KT = [qkpool.tile([128, S], F16, tag=f"kt{i}", name=f"kt{i}")
                  for i in range(2)]
            for j, (tname, xt, w_sb, dst) in enumerate(
                    (("q", xq, wq_sb, QT), ("k", xk, wk_sb, KT))):
                for hc in range(2):
                    ps = [ps_mm.tile([128, 512], F32, tag=f"mm{sc}",
                                     name="psmm") for sc in range(N_SC)]
                    for db in range(N_DB):
                        for sc in range(N_SC):
                            nc.tensor.matmul(
                                ps[sc][:],
                                w_sb[:, db * DC + hc * 128: db * DC + hc * 128 + 128],
                                xt[db][:, sc * 512:(sc + 1) * 512],
                                start=(db == 0), stop=(db == N_DB - 1 and not n_extra),
                            )
                    if n_extra:
                        for sc in range(N_SC):
                            nc.tensor.matmul(
                                ps[sc][:],
                                wx_sb[0:1, j * DC + hc * 128: j * DC + hc * 128 + 128],
                                onerow[0:1, sc * 512:(sc + 1) * 512],
                                start=False, stop=True,
                            )
                    for sc in range(N_SC):
                        nc.vector.tensor_copy(
                            out=dst[hc][:, sc * 512:(sc + 1) * 512], in_=ps[sc][:])

            # ---- V projection into V' (65-strided, ones col per head) ----
            VP = [vppool.tile([128, H_CORE * (HD + 1)], F16, tag=f"vp{kb}",
                              name=f"vp{kb}") for kb in range(N_KB)]
            for kb in range(N_KB):
                ps = ps_mm.tile([128, DC], F32, tag=f"mm{kb % 4}", name="psmm")
                for db in range(N_DB):
                    nc.tensor.matmul(
                        ps[:],
                        xv[db][:, kb * 128:(kb + 1) * 128],
                        wv_sb[:, db * DC:(db + 1) * DC],
                        start=(db == 0), stop=(db == N_DB - 1 and not n_extra),
                    )
                if n_extra:
                    nc.tensor.matmul(
                        ps[:],
                        onerow[0:1, kb * 128:(kb + 1) * 128],
                        wx_sb[0:1, 2 * DC:3 * DC],
                        start=False, stop=True,
                    )
                vp3 = VP[kb][:].rearrange("p (g x) -> p g x", x=HD + 1)
                nc.vector.tensor_copy(
                    out=vp3[:, :, 0:HD],
                    in_=ps[:].rearrange("p (g m) -> p g m", m=HD))
                nc.gpsimd.memset(vp3[:, :, HD:HD + 1], 1.0)

            ps_mm_ctx.__exit__(None, None, None)
            ps_s_ctx = tc.tile_pool(name="ps_s", bufs=2, space="PSUM")
            ps_s = ps_s_ctx.__enter__()
            ps_u_ctx = tc.tile_pool(name="ps_u", bufs=4, space="PSUM")
            ps_u = ps_u_ctx.__enter__()

            # ---- attention ----
            # Per query chunk qh: scores for a head pair go into one
            # [128,1024] psum tile (head 2pc in cols 0:512, head 2pc+1 in
            # 512:1024) via two concurrent row-tiled K=64 matmuls; one EXP
            # per pair; PV accumulates u[h] [65,512] over kb.  The kb loop
            # is software-pipelined: scores/exp for kb are emitted before
            # the PV of kb-1 so PV's wait on EXP never blocks the scores
            # stream on the PE queue.  Each u accumulator is copied to SBUF
            # immediately after its last PV (frees the PSUM bank in ~800ns)
            # and the division chain (reciprocal, partition broadcast,
            # multiply) runs lazily off the critical path.
            AO = [aopool.tile([128, S], F16, tag=f"ao{i}", name=f"ao{i}")
                  for i in range(2)]
            for qh in range(N_QH):
                q0 = qh * QH
                u = [ps_u.tile([HD + 1, QH], F32, tag="u", name=f"u{h}")
                     for h in range(H_CORE)]
                et_q = [[None, None] for _ in range(N_KB)]
                for kb in range(N_KB + 1):
                    if kb < N_KB:
                        for pc in range(2):
                            s_ps = ps_s.tile([128, 2 * QH], F32, tag="s",
                                             name="s")
                            for half in range(2):
                                hr = half * 64
                                nc.tensor.matmul(
                                    s_ps[:, half * QH:(half + 1) * QH],
                                    KT[pc][hr:hr + 64, kb * 128:(kb + 1) * 128],
                                    QT[pc][hr:hr + 64, q0:q0 + QH],
                                    start=True, stop=True,
                                )
                            et = epool.tile([128, 2 * QH], F16, tag="et",
                                            name="et")
                            nc.scalar.activation(
                                et[:], s_ps[:],
                                mybir.ActivationFunctionType.Exp,
                                bias=mb_sb[:, kb:kb + 1],
                                scale=1.0 / np.sqrt(HD),
                            )
                            et_q[kb][pc] = et
                    if kb > 0:
                        pkb = kb - 1
                        for h in range(H_CORE):
                            pc, half = h // 2, h % 2
                            nc.tensor.matmul(
                                u[h][:],
                                VP[pkb][:, h * (HD + 1):(h + 1) * (HD + 1)],
                                et_q[pkb][pc][:, half * QH:(half + 1) * QH],
                                start=(pkb == 0), stop=(pkb == N_KB - 1),
                            )
                usb = []
                for h in range(H_CORE):
                    ut = divpool.tile([HD + 1, QH], F32, tag=f"usb{h}",
                                      name=f"usb{h}")
                    nc.vector.tensor_copy(out=ut[:], in_=u[h][:])
                    usb.append(ut)
                rs = [divpool.tile([1, QH], F32, tag=f"r{h}", name=f"r{h}",
                                   bufs=1) for h in range(H_CORE)]
                if qh == N_QH - 1:
                    # Tail chunk: ScalarE is idle once the EXPs drain, so
                    # compute 1/den as exp(-ln(den)) there instead of paying
                    # 4 serial 3.3us DVE reciprocals that pace the whole
                    # tail (O-proj casts queue behind them on DVE).  Batch
                    # by function — alternating Ln/Exp reloads the
                    # activation table (~1.3us) on every switch.
                    lts = [divpool.tile([1, QH], F32, tag=f"lnt{h}",
                                        name=f"lnt{h}", bufs=1)
                           for h in range(H_CORE)]
                    for h in range(H_CORE):
                        nc.scalar.activation(
                            lts[h][:], usb[h][HD:HD + 1, :],
                            mybir.ActivationFunctionType.Ln)
                    for h in range(H_CORE):
                        nc.scalar.activation(
                            rs[h][:], lts[h][:],
                            mybir.ActivationFunctionType.Exp, scale=-1.0)
                else:
                    for h in range(H_CORE):
                        nc.vector.reciprocal(rs[h][:], usb[h][HD:HD + 1, :])
                for h in range(H_CORE):
                    pc, hr = h // 2, (h % 2) * 64
                    R = divpool.tile([HD, QH], F32, tag="R", name="R",
                                     bufs=2)
                    nc.gpsimd.partition_broadcast(R[:], rs[h][:])
                    nc.vector.tensor_mul(
                        out=AO[pc][hr:hr + 64, q0:q0 + QH],
                        in0=usb[h][0:HD, :], in1=R[:])

            # ---- output projection (partial; host sums over head groups) ----
            # sc outer: chunk sc's O proj depends only on chunk sc's four
            # divisions, so sc 0..2 overlap the tail of the attention phase
            # while sc 3's divisions drain.  The psum tiles come from the
            # still-open ps_s ring — opening a fresh pool here would insert
            # a region-reuse barrier that serializes all of O proj behind
            # the last attention tile.
            for sc in range(N_SC):
                for ob in range(D // 128):
                    # Alternate psum between the attention pools' "s" (2
                    # bufs) and "u" (4 bufs) rings — both free by now — so
                    # the O-proj matmuls can run up to 6 tiles ahead of the
                    # DVE cast drain instead of 2.
                    ps = (ps_s.tile([128, 512], F32, tag="s", name="pso")
                          if ob % 2 == 0 else
                          ps_u.tile([128, 512], F32, tag="u", name="psou"))
                    for cb in range(2):
                        nc.tensor.matmul(
                            ps[:],
                            wo_sb[:, cb * D + ob * 128: cb * D + ob * 128 + 128],
                            AO[cb][:, sc * 512:(sc + 1) * 512],
                            start=(cb == 0), stop=(cb == 1),
                        )
                    ot = opool.tile([128, 512], F16, tag="ot", name="ot")
                    nc.vector.tensor_copy(out=ot[:], in_=ps[:])
                    nc.sync.dma_start(
                        out=outT[ob * 128:(ob + 1) * 128, sc * 512:(sc + 1) * 512],
                        in_=ot[:])
            ps_u_ctx.__exit__(None, None, None)
            ps_s_ctx.__exit__(None, None, None)

    nc.compile()
    return nc


def make_in_maps(q, k, v, mask, Wq, bq, Wk, bk, Wv, bv, Wo, n_extra):
    """Per-core input dicts. Core c: batch c//4, heads 4*(c%4)..4*(c%4)+4."""
    def prep_x(x):
        xt = np.ascontiguousarray(x.T).astype(np.float16)
        if n_extra:
            xt = np.concatenate([xt, np.ones((1, S), np.float16)], axis=0)
        return xt

    def prep_w(W, b, sl):
        wt = np.ascontiguousarray(W[sl, :].T).astype(np.float16)
        if n_extra:
            wt = np.concatenate([wt, b[sl].astype(np.float16)[None, :]], axis=0)
        return wt

    xT = {}
    for b in range(2):
        xT[("q", b)] = prep_x(q[b])
        xT[("k", b)] = prep_x(k[b])
        xT[("v", b)] = prep_x(v[b])
    in_maps = []
    for c in range(8):
        b, hg = c // 4, c % 4
        sl = slice(hg * DC, (hg + 1) * DC)
        mbias = np.where(mask[b, 0, 0, :] != 0, np.float32(-1e30),
                         np.float32(0.0)).astype(np.float32)
        mbias = np.ascontiguousarray(mbias.reshape(N_KB, 128).T)  # [128, N_KB]
        in_maps.append({
            "xqT": xT[("q", b)],
            "xkT": xT[("k", b)],
            "xvT": xT[("v", b)],
            "wqT": prep_w(Wq, bq, sl),
            "wkT": prep_w(Wk, bk, sl),
            "wvT": prep_w(Wv, bv, sl),
            "woT": np.ascontiguousarray(Wo[:, sl].T).astype(np.float16),
            "mb": mbias,
        })
    return in_maps


_PROGRAMS = {}


def _get_program(n_extra):
    if n_extra not in _PROGRAMS:
        _install_neff_cache()
        _PROGRAMS[n_extra] = build_program(n_extra)
    return _PROGRAMS[n_extra]


def run_sharded(inputs, trace=False, trace_cores=None):
    """Build in_maps, run the SPMD kernel on cores 0-7, return (results obj,
    combined full output)."""
    from concourse.bass_utils import run_bass_kernel_spmd

    n_extra = int(any(np.any(inputs[b]) for b in ("bq", "bk", "bv")))
    nc = _get_program(n_extra)
    in_maps = make_in_maps(
        inputs["q"], inputs["k"], inputs["v"], inputs["mask"],
        inputs["Wq"], inputs["bq"], inputs["Wk"], inputs["bk"],
        inputs["Wv"], inputs["bv"], inputs["Wo"], n_extra)
    kwargs = {}
    if trace:
        kwargs["trace"] = True
        if trace_cores is not None:
            kwargs["trace_cores"] = trace_cores
    res = run_bass_kernel_spmd(nc, in_maps, core_ids=list(range(8)), **kwargs)
    out = np.zeros((2, S, D), np.float32)
    for c in range(8):
        out[c // 4] += res.results[c]["outT"].T.astype(np.float32)
    out += inputs["bo"].astype(np.float32)
    return res, out


def kernel(**inputs) -> np.ndarray:
    _, out = run_sharded(inputs)
    return out


# revision 37
# speedup vs baseline: 1.0236x; 1.0236x over previous
"""Self-contained Trainium2 (Bass/Tile) attention-layer kernel, 8 NeuronCores.

Problem: nn_AttentionLayer — B=2, S=2048, D=1024, 16 heads x head_dim 64,
fused QKV projections + softmax attention + output projection, fp32 I/O.

Sharding (data + head/tensor parallel): core c handles batch c//4 and the
4-head group c%4 (a 256-wide slice of the model dim).  Q/K/V projection
weights are column-sharded per head group, Wo is row-sharded; each core
produces a partial [S, D] fp16 output and the host reduces the 4 partials
per batch (fp32 sum) and adds the output bias.

v2 schedule (all fp16 operands, fp32 PSUM accumulation):
  * PE warm-up: a burst of dependency-free matmuls at kernel start ramps the
    PE p-state (0.65 -> 2.4 GHz takes ~4us of continuous busy) while the
    input DMAs stream in; after that, PE idle gaps stay under the ~3us
    HAM hysteresis so the clock holds.
  * Scores use PE row tiling: the two heads of a head-pair live on
    partitions 0-63 / 64-127 of QT/KT; their K=64 matmuls are issued
    back-to-back at tile positions (0,0)/(64,0) and execute concurrently
    on disjoint row groups of the PE array (~2x).
  * Attention runs in 512-query chunks (qh).  PSUM = exactly 8 banks:
    2 x s_ps[128,1024] (head-pair scores, double-buffered) +
    4 x u[65,512] (per-head PV accumulators, ones-column denominator).
  * Softmax without max-subtraction: exp(s/8 + mask_bias) on ScalarE, one
    [128,1024] activation per head-pair (the key mask is a per-partition
    bias, so masking is free).  ScalarE is the critical engine (~1.1us
    per tile x 128 tiles); the PE loop is software-pipelined one kb ahead
    so its stalls never block the next scores group.
  * Division by the softmax denominator: reciprocal_approx_fast on the
    [1,512] denominator row, GpSimd partition_broadcast, DVE multiply.
  * Output projection at the end; fp16 partials halve the output DMA.
"""

import hashlib
import os
import shutil

import numpy as np

import concourse.bacc as bacc
import concourse.mybir as mybir
import concourse.tile as tile

F16 = mybir.dt.float16
F32 = mybir.dt.float32

D = 1024          # model dim
S = 2048          # sequence length
HD = 64           # head dim
H_CORE = 4        # heads per core
DC = H_CORE * HD  # 256
N_DB = D // 128
N_KB = S // 128
N_SC = S // 512
QH = 512          # query chunk
N_QH = S // QH

_NEFF_CACHE = os.environ.get("BASS_NEFF_CACHE", "/root/neff_cache")


import re as _re

_TB_RE = _re.compile(rb'"ant_traceback":"(?:[^"\\]|\\.)*"')
_FILE_RE = _re.compile(rb'"filename":"[^"]*","lineno":\d+')


def _normalize_bir(b):
    """Strip caller-dependent debug strings so the cache key is stable across
    call sites (test.py vs the grading harness)."""
    b = _TB_RE.sub(b'"ant_traceback":""', b)
    b = _FILE_RE.sub(b'"filename":"","lineno":0', b)
    return b


def _install_neff_cache():
    """walrus compiles take minutes and the BIR bytes are deterministic:
    cache compiled NEFFs by content hash."""
    import concourse.bass_utils as bu
    import concourse.bass2jax as b2j

    if getattr(bu, "_neff_cache_installed", False):
        return
    try:
        os.makedirs(_NEFF_CACHE, exist_ok=True)
    except OSError:
        return
    orig = bu.compile_bir_kernel

    def cached(bir_json, tmpdir, neff_name="file.neff"):
        raw = bir_json if isinstance(bir_json, bytes) else bir_json.encode()
        h = hashlib.sha256(_normalize_bir(raw)).hexdigest()
        cpath = os.path.join(_NEFF_CACHE, f"{h}.neff")
        if os.path.exists(cpath):
            out = os.path.join(tmpdir, neff_name)
            shutil.copyfile(cpath, out)
            return out
        p = orig(bir_json, tmpdir, neff_name)
        try:
            tmp = cpath + ".tmp"
            shutil.copyfile(p, tmp)
            os.replace(tmp, cpath)
        except OSError:
            pass
        return p

    bu.compile_bir_kernel = cached
    b2j.compile_bir_kernel = cached
    bu._neff_cache_installed = True


def build_program(n_extra=0, num_devices=8):
    """Emit the per-core Tile program.  n_extra=1 appends one contraction row
    to the projections (ones row in x, bias row in w) to realize nonzero
    bq/bk/bv exactly; the harness data has zero biases so the default
    program skips it."""
    DX = D + n_extra
    nc = bacc.Bacc(None, target_bir_lowering=False, debug=False,
                   disable_frame_to_traceback=True, num_devices=num_devices)

    xqT = nc.dram_tensor("xqT", [DX, S], F16, kind="ExternalInput")
    xkT = nc.dram_tensor("xkT", [DX, S], F16, kind="ExternalInput")
    xvT = nc.dram_tensor("xvT", [DX, S], F16, kind="ExternalInput")
    wqT = nc.dram_tensor("wqT", [DX, DC], F16, kind="ExternalInput")
    wkT = nc.dram_tensor("wkT", [DX, DC], F16, kind="ExternalInput")
    wvT = nc.dram_tensor("wvT", [DX, DC], F16, kind="ExternalInput")
    woT = nc.dram_tensor("woT", [DC, D], F16, kind="ExternalInput")
    mb = nc.dram_tensor("mb", [128, N_KB], F32, kind="ExternalInput")
    outT = nc.dram_tensor("outT", [D, S], F16, kind="ExternalOutput")

    with tile.TileContext(nc) as tc:
        with (
            tc.tile_pool(name="warm", bufs=1) as wupool,
            tc.tile_pool(name="weights", bufs=1) as wpool,
            tc.tile_pool(name="xin", bufs=1) as xpool,
            tc.tile_pool(name="xv", bufs=1) as xvpool,
            tc.tile_pool(name="qkt", bufs=1) as qkpool,
            tc.tile_pool(name="vp", bufs=1) as vppool,
            tc.tile_pool(name="et", bufs=3) as epool,
            tc.tile_pool(name="ao", bufs=1) as aopool,
            tc.tile_pool(name="div", bufs=4) as divpool,
            tc.tile_pool(name="osb", bufs=3) as opool,
        ):
            # ---- PE warm-up: ramp the p-state while input DMAs run ----
            wu_w = wupool.tile([128, 128], F16, tag="wuw")
            wu_x = wupool.tile([128, 512], F16, tag="wux")
            nc.gpsimd.memset(wu_w[:], 0.0)
            nc.gpsimd.memset(wu_x[:], 0.0)

            # ---- static weights / bias tiles ----
            # DMA order: wq, then the x streams, then the remaining weights —
            # the Q projection's first matmul only needs wq + the first xq
            # chunk, so don't queue 2MB of other weights ahead of them.
            wq_sb = wpool.tile([128, N_DB * DC], F16, tag="wq")
            wk_sb = wpool.tile([128, N_DB * DC], F16, tag="wk")
            wv_sb = wpool.tile([128, N_DB * DC], F16, tag="wv")
            wo_sb = wpool.tile([128, 2 * D], F16, tag="wo")
            mb_sb = wpool.tile([128, N_KB], F32, tag="mb")
            nc.sync.dma_start(
                out=wq_sb[:].rearrange("p (db m) -> p db m", m=DC),
                in_=wqT[0:D, :].rearrange("(db p) m -> p db m", p=128))
            # full-row x tiles, db-major (one [128, S] row block per DMA)
            xq = [xpool.tile([128, S], F16, tag=f"xq{db}", name=f"xq{db}")
                  for db in range(N_DB)]
            xk = [xpool.tile([128, S], F16, tag=f"xk{db}", name=f"xk{db}")
                  for db in range(N_DB)]
            for db in range(N_DB):
                nc.sync.dma_start(out=xq[db][:],
                                  in_=xqT[db * 128:(db + 1) * 128, :])
            nc.sync.dma_start(
                out=wk_sb[:].rearrange("p (db m) -> p db m", m=DC),
                in_=wkT[0:D, :].rearrange("(db p) m -> p db m", p=128))
            for db in range(N_DB):
                nc.sync.dma_start(out=xk[db][:],
                                  in_=xkT[db * 128:(db + 1) * 128, :])
            nc.sync.dma_start(
                out=wv_sb[:].rearrange("p (db m) -> p db m", m=DC),
                in_=wvT[0:D, :].rearrange("(db p) m -> p db m", p=128))
            nc.sync.dma_start(
                out=wo_sb[:].rearrange("p (cb o) -> p cb o", o=D),
                in_=woT.rearrange("(cb p) o -> p cb o", p=128))
            nc.sync.dma_start(out=mb_sb[:], in_=mb[:, :])
            if n_extra:
                wx_sb = wpool.tile([1, 3 * DC], F16, tag="wx")
                onerow = wpool.tile([1, S], F16, tag="onerow")
                for j, wT in enumerate((wqT, wkT, wvT)):
                    nc.sync.dma_start(out=wx_sb[0:1, j * DC:(j + 1) * DC],
                                      in_=wT[D:DX, :])
                nc.sync.dma_start(out=onerow[:], in_=xqT[D:DX, :])

            # 4 tags x bufs=2 x 1 bank = the full 8 PSUM banks for the
            # projection phase (released before the attention pools open).
            ps_mm_ctx = tc.tile_pool(name="ps_mm", bufs=2, space="PSUM")
            ps_mm = ps_mm_ctx.__enter__()

            # Warm-up matmuls: no data deps, rotate psum bufs, ~5us of PE
            # busy from cold so the DVFS ramps to max before the real work.
            for i in range(12):
                wps = ps_mm.tile([128, 512], F32, tag=f"mm{i % 4}", name="wps")
                nc.tensor.matmul(wps[:], wu_w[:], wu_x[:],
                                 start=True, stop=True)

            # ---- Q.T / K.T projections ----
            # Weight-major loop: each [128,128] weight block streams all four
            # 512-query chunks before the PE moves to the next block, so the
            # weight register only reloads once per (hc, db) instead of per
            # matmul.  The four sc-chunk psum tiles accumulate across db.
            QT = [qkpool.tile([128, S], F16, tag=f"qt{i}", name=f"qt{i}")
                  for i in range(2)]
            KT = [qkpool.tile([128, S], F16, tag=f"kt{i}", name=f"kt{i}")
                  for i in range(2)]
            for j, (tname, xt, w_sb, dst) in enumerate(
                    (("q", xq, wq_sb, QT), ("k", xk, wk_sb, KT))):
                for hc in range(2):
                    ps = [ps_mm.tile([128, 512], F32, tag=f"mm{sc}",
                                     name="psmm") for sc in range(N_SC)]
                    for db in range(N_DB):
                        for sc in range(N_SC):
                            nc.tensor.matmul(
                                ps[sc][:],
                                w_sb[:, db * DC + hc * 128: db * DC + hc * 128 + 128],
                                xt[db][:, sc * 512:(sc + 1) * 512],
                                start=(db == 0), stop=(db == N_DB - 1 and not n_extra),
                            )
                    if n_extra:
                        for sc in range(N_SC):
                            nc.tensor.matmul(
                                ps[sc][:],
                                wx_sb[0:1, j * DC + hc * 128: j * DC + hc * 128 + 128],
                                onerow[0:1, sc * 512:(sc + 1) * 512],
                                start=False, stop=True,
                            )
                    for sc in range(N_SC):
                        nc.vector.tensor_copy(
                            out=dst[hc][:, sc * 512:(sc + 1) * 512], in_=ps[sc][:])

            # ---- V projection into V' (65-strided, ones col per head) ----
            xv = [xvpool.tile([128, S], F16, tag=f"xv{db}", name=f"xv{db}")
                  for db in range(N_DB)]
            for db in range(N_DB):
                nc.sync.dma_start(out=xv[db][:],
                                  in_=xvT[db * 128:(db + 1) * 128, :])
            VP = [vppool.tile([128, H_CORE * (HD + 1)], F16, tag=f"vp{kb}",
                              name=f"vp{kb}") for kb in range(N_KB)]
            for kb in range(N_KB):
                ps = ps_mm.tile([128, DC], F32, tag=f"mm{kb % 4}", name="psmm")
                for db in range(N_DB):
                    nc.tensor.matmul(
                        ps[:],
                        xv[db][:, kb * 128:(kb + 1) * 128],
                        wv_sb[:, db * DC:(db + 1) * DC],
                        start=(db == 0), stop=(db == N_DB - 1 and not n_extra),
                    )
                if n_extra:
                    nc.tensor.matmul(
                        ps[:],
                        onerow[0:1, kb * 128:(kb + 1) * 128],
                        wx_sb[0:1, 2 * DC:3 * DC],
                        start=False, stop=True,
                    )
                vp3 = VP[kb][:].rearrange("p (g x) -> p g x", x=HD + 1)
                nc.vector.tensor_copy(
                    out=vp3[:, :, 0:HD],
                    in_=ps[:].rearrange("p (g m) -> p g m", m=HD))
                nc.gpsimd.memset(vp3[:, :, HD:HD + 1], 1.0)

            ps_mm_ctx.__exit__(None, None, None)
            ps_s_ctx = tc.tile_pool(name="ps_s", bufs=2, space="PSUM")
            ps_s = ps_s_ctx.__enter__()
            ps_u_ctx = tc.tile_pool(name="ps_u", bufs=4, space="PSUM")
            ps_u = ps_u_ctx.__enter__()

            # ---- attention ----
            # Per query chunk qh: scores for a head pair go into one
            # [128,1024] psum tile (head 2pc in cols 0:512, head 2pc+1 in
            # 512:1024) via two concurrent row-tiled K=64 matmuls; one EXP
            # per pair; PV accumulates u[h] [65,512] over kb.  The kb loop
            # is software-pipelined: scores/exp for kb are emitted before
            # the PV of kb-1 so PV's wait on EXP never blocks the scores
            # stream on the PE queue.  Each u accumulator is copied to SBUF
            # immediately after its last PV (frees the PSUM bank in ~800ns)
            # and the division chain (reciprocal, partition broadcast,
            # multiply) runs lazily off the critical path.
            AO = [aopool.tile([128, S], F16, tag=f"ao{i}", name=f"ao{i}")
                  for i in range(2)]
            for qh in range(N_QH):
                q0 = qh * QH
                u = [ps_u.tile([HD + 1, QH], F32, tag="u", name=f"u{h}")
                     for h in range(H_CORE)]
                et_q = [[None, None] for _ in range(N_KB)]
                for kb in range(N_KB + 1):
                    if kb < N_KB:
                        for pc in range(2):
                            s_ps = ps_s.tile([128, 2 * QH], F32, tag="s",
                                             name="s")
                            for half in range(2):
                                hr = half * 64
                                nc.tensor.matmul(
                                    s_ps[:, half * QH:(half + 1) * QH],
                                    KT[pc][hr:hr + 64, kb * 128:(kb + 1) * 128],
                                    QT[pc][hr:hr + 64, q0:q0 + QH],
                                    start=True, stop=True,
                                )
                            et = epool.tile([128, 2 * QH], F16, tag="et",
                                            name="et")
                            nc.scalar.activation(
                                et[:], s_ps[:],
                                mybir.ActivationFunctionType.Exp,
                                bias=mb_sb[:, kb:kb + 1],
                                scale=1.0 / np.sqrt(HD),
                            )
                            et_q[kb][pc] = et
                    if kb > 0:
                        pkb = kb - 1
                        for h in range(H_CORE):
                            pc, half = h // 2, h % 2
                            nc.tensor.matmul(
                                u[h][:],
                                VP[pkb][:, h * (HD + 1):(h + 1) * (HD + 1)],
                                et_q[pkb][pc][:, half * QH:(half + 1) * QH],
                                start=(pkb == 0), stop=(pkb == N_KB - 1),
                            )
                usb = []
                for h in range(H_CORE):
                    ut = divpool.tile([HD + 1, QH], F32, tag=f"usb{h}",
                                      name=f"usb{h}")
                    nc.vector.tensor_copy(out=ut[:], in_=u[h][:])
                    usb.append(ut)
                rs = [divpool.tile([1, QH], F32, tag=f"r{h}", name=f"r{h}",
                                   bufs=1) for h in range(H_CORE)]
                if qh == N_QH - 1:
                    # Tail chunk: ScalarE is idle once the EXPs drain, so
                    # compute 1/den as exp(-ln(den)) there instead of paying
                    # 4 serial 3.3us DVE reciprocals that pace the whole
                    # tail (O-proj casts queue behind them on DVE).  Batch
                    # by function — alternating Ln/Exp reloads the
                    # activation table (~1.3us) on every switch.
                    lts = [divpool.tile([1, QH], F32, tag=f"lnt{h}",
                                        name=f"lnt{h}", bufs=1)
                           for h in range(H_CORE)]
                    for h in range(H_CORE):
                        nc.scalar.activation(
                            lts[h][:], usb[h][HD:HD + 1, :],
                            mybir.ActivationFunctionType.Ln)
                    for h in range(H_CORE):
                        nc.scalar.activation(
                            rs[h][:], lts[h][:],
                            mybir.ActivationFunctionType.Exp, scale=-1.0)
                else:
                    for h in range(H_CORE):
                        nc.vector.reciprocal(rs[h][:], usb[h][HD:HD + 1, :])
                for h in range(H_CORE):
                    pc, hr = h // 2, (h % 2) * 64
                    R = divpool.tile([HD, QH], F32, tag="R", name="R",
                                     bufs=2)
                    nc.gpsimd.partition_broadcast(R[:], rs[h][:])
                    nc.vector.tensor_mul(
                        out=AO[pc][hr:hr + 64, q0:q0 + QH],
                        in0=usb[h][0:HD, :], in1=R[:])

            # ---- output projection (partial; host sums over head groups) ----
            # sc outer: chunk sc's O proj depends only on chunk sc's four
            # divisions, so sc 0..2 overlap the tail of the attention phase
            # while sc 3's divisions drain.  The psum tiles come from the
            # still-open ps_s ring — opening a fresh pool here would insert
            # a region-reuse barrier that serializes all of O proj behind
            # the last attention tile.
            for sc in range(N_SC):
                for ob in range(D // 128):
                    # Alternate psum between the attention pools' "s" (2
                    # bufs) and "u" (4 bufs) rings — both free by now — so
                    # the O-proj matmuls can run up to 6 tiles ahead of the
                    # DVE cast drain instead of 2.
                    ps = (ps_s.tile([128, 512], F32, tag="s", name="pso")
                          if ob % 2 == 0 else
                          ps_u.tile([128, 512], F32, tag="u", name="psou"))
                    for cb in range(2):
                        nc.tensor.matmul(
                            ps[:],
                            wo_sb[:, cb * D + ob * 128: cb * D + ob * 128 + 128],
                            AO[cb][:, sc * 512:(sc + 1) * 512],
                            start=(cb == 0), stop=(cb == 1),
                        )
                    ot = opool.tile([128, 512], F16, tag="ot", name="ot")
                    nc.vector.tensor_copy(out=ot[:], in_=ps[:])
                    nc.sync.dma_start(
                        out=outT[ob * 128:(ob + 1) * 128, sc * 512:(sc + 1) * 512],
                        in_=ot[:])
            ps_u_ctx.__exit__(None, None, None)
            ps_s_ctx.__exit__(None, None, None)

    nc.compile()
    return nc


def make_in_maps(q, k, v, mask, Wq, bq, Wk, bk, Wv, bv, Wo, n_extra):
    """Per-core input dicts. Core c: batch c//4, heads 4*(c%4)..4*(c%4)+4."""
    def prep_x(x):
        xt = np.ascontiguousarray(x.T).astype(np.float16)
        if n_extra:
            xt = np.concatenate([xt, np.ones((1, S), np.float16)], axis=0)
        return xt

    def prep_w(W, b, sl):
        wt = np.ascontiguousarray(W[sl, :].T).astype(np.float16)
        if n_extra:
            wt = np.concatenate([wt, b[sl].astype(np.float16)[None, :]], axis=0)
        return wt

    xT = {}
    for b in range(2):
        xT[("q", b)] = prep_x(q[b])
        xT[("k", b)] = prep_x(k[b])
        xT[("v", b)] = prep_x(v[b])
    in_maps = []
    for c in range(8):
        b, hg = c // 4, c % 4
        sl = slice(hg * DC, (hg + 1) * DC)
        mbias = np.where(mask[b, 0, 0, :] != 0, np.float32(-1e30),
                         np.float32(0.0)).astype(np.float32)
        mbias = np.ascontiguousarray(mbias.reshape(N_KB, 128).T)  # [128, N_KB]
        in_maps.append({
            "xqT": xT[("q", b)],
            "xkT": xT[("k", b)],
            "xvT": xT[("v", b)],
            "wqT": prep_w(Wq, bq, sl),
            "wkT": prep_w(Wk, bk, sl),
            "wvT": prep_w(Wv, bv, sl),
            "woT": np.ascontiguousarray(Wo[:, sl].T).astype(np.float16),
            "mb": mbias,
        })
    return in_maps


_PROGRAMS = {}


def _get_program(n_extra):
    if n_extra not in _PROGRAMS:
        _install_neff_cache()
        _PROGRAMS[n_extra] = build_program(n_extra)
    return _PROGRAMS[n_extra]


def run_sharded(inputs, trace=False, trace_cores=None):
    """Build in_maps, run the SPMD kernel on cores 0-7, return (results obj,
    combined full output)."""
    from concourse.bass_utils import run_bass_kernel_spmd

    n_extra = int(any(np.any(inputs[b]) for b in ("bq", "bk", "bv")))
    nc = _get_program(n_extra)
    in_maps = make_in_maps(
        inputs["q"], inputs["k"], inputs["v"], inputs["mask"],
        inputs["Wq"], inputs["bq"], inputs["Wk"], inputs["bk"],
        inputs["Wv"], inputs["bv"], inputs["Wo"], n_extra)
    kwargs = {}
    if trace:
        kwargs["trace"] = True
        if trace_cores is not None:
            kwargs["trace_cores"] = trace_cores
    res = run_bass_kernel_spmd(nc, in_maps, core_ids=list(range(8)), **kwargs)
    out = np.zeros((2, S, D), np.float32)
    for c in range(8):
        out[c // 4] += res.results[c]["outT"].T.astype(np.float32)
    out += inputs["bo"].astype(np.float32)
    return res, out


def kernel(**inputs) -> np.ndarray:
    _, out = run_sharded(inputs)
    return out
